# revision 5
# baseline (speedup 1.0000x reference)
"""Trainium2 Bass kernel for nn_DropLearner (gnn_message_passing).

aug_edge_weight = sigmoid((logit(eps) + MLP([head|tail|rel])) / T)

Strategy (8 NeuronCores, data-parallel over edges):
  - Edges sharded 62500/core after a global sort by head, padded to
    62976 slots = 30 groups x 2048 + 1 group x 1536.
  - The SWDGE fixed cost (~1 us per indirect-DMA instruction on the
    Pool engine) is the kernel's floor, and every slot column needs its
    OTHER endpoint single-gathered, so the floor is ~1 Pool instruction
    per column per endpoint unless an endpoint avoids gathers entirely.
  - HEAD side avoids Pool almost completely: per-core head values have
    multiplicity ~5 (62500 edges over ~12500 node ids), so head-uniques
    are packed into multiplicity-class cells (class m = 2..9, filled by
    a demotion ladder from m=9 down; a cell = 128 uniques x m edges).
    A per-core class table tabc holds each unique's fp32 row ONCE in
    cell order; the m columns of a cell all read the SAME 128 rows by
    plain sequential HWDGE DMA -- the device expands duplicates by
    re-reading DRAM m times, with ZERO Pool instructions and no index
    tables. 473/492 columns (~97% of head edges) are covered; the rest
    are single-gathered.
  - TAIL side (uniform over all nodes, no exploitable locality once
    slots are bound by the head layout) is single-gathered: one
    indirect-DMA instruction per column (one int32 index per output
    partition streams one 512B row).
  - Pool instruction count: 492 tail + ~19 head singles = 511
    (vs 654 for run/pair-matched gathers on both endpoints).
  - Gathered edge-major tiles are transposed feature-major on the
    TensorEngine (fp32 128x128 blocks into PSUM, copied out via
    DVE/ACT).
  - MLP: h.T[192, 512] accumulated in PSUM from 3 matmuls per 96-half:
    W1h.T @ headT + W1t.T @ tailT + Rb.T @ onehot(type); relu-copied to
    SBUF; weight = W2 @ h via matmuls into packed PSUM rows
    (tile_position col-packing).
  - Per-edge weights staged to DRAM, re-read as [128, 492] for bulk
    gating (Ln/sigmoid on the scalar engine).
"""
import sys
sys.path.insert(0, "/opt/trn_rl_repo")

import contextlib
import numpy as np

import concourse.bacc as bacc
import concourse.bass as bass
import concourse.mybir as mybir
import concourse.tile as tile
from concourse.bass_utils import run_bass_kernel_spmd

# ---- problem constants (hardcoded per task contract) ----
N_NODES = 100000
D = 128           # node dim
N_REL = 32
E = 500000
H = 192           # 3 * mlp_dim
TEMP = 0.5
BIAS = 1e-4

NCORES = 8
EC = E // NCORES              # 62500 edges per core
NG = 31
GCOLS = [16] * 30 + [12]      # j-columns per group (last group trimmed)
GCH = [4] * 30 + [3]          # 512-edge chunks per group
F = sum(GCOLS)                # 492 total columns
EP = 128 * F                  # 62976 slots per core
SOFF = [g * 2048 for g in range(NG)]          # slot offset of each group
COFF = np.cumsum([0] + GCOLS).tolist()        # global col offset per group

BF16 = mybir.dt.bfloat16
F16 = mybir.dt.float16
F32 = mybir.dt.float32
I32 = mybir.dt.int32

_CACHE = {}


def _regions(h16, t16, h8, t8, qh, qt, h3, t3, ph, pt, hd, td):
    """Column regions per endpoint: (lo, hi, L, kind). kind "run" =
    one gather streams L consecutive rows; kind "tri" = 4-col cells of
    a 3-row run plus one single-fillable column; kind "dup" = gather
    the even column only, the odd column is a DVE copy of it (same node
    id). Wider (16/8) regions first so every region start is aligned
    to its width and no span straddles a 16-column group boundary."""
    A = 16 * (h16 + t16)
    B = A + 8 * (h8 + t8)
    Q = B + 4 * (qh + qt)
    T = Q + 4 * (h3 + t3)
    R = T + 2 * (ph + pt)
    head = [(0, 16 * h16, 16, "run"),
            (A, A + 8 * h8, 8, "run"),
            (B, B + 4 * qh, 4, "run"),
            (Q, Q + 4 * h3, 4, "tri"),
            (T, T + 2 * ph, 2, "run"),
            (R, R + 2 * hd, 2, "dup")]
    tail = [(16 * h16, A, 16, "run"),
            (A + 8 * h8, B, 8, "run"),
            (B + 4 * qh, B + 4 * (qh + qt), 4, "run"),
            (Q + 4 * h3, T, 4, "tri"),
            (T + 2 * ph, T + 2 * ph + 2 * pt, 2, "run"),
            (R + 2 * hd, R + 2 * hd + 2 * td, 2, "dup")]
    return head, tail


def _build_program(cap16, cls_caps):
    """cap16: head 16-run blocks (one indirect instruction per block of
    128 runs x 16 consecutive node ids). cls_caps[m-2] = number of
    128-unique cells of head-multiplicity class m (m=2..9): each cell
    spans m adjacent columns that all read the SAME 128 rows of the
    class table tabc by plain sequential DMA (the device re-reads the
    rows m times = on-device dup expansion, zero Pool instructions).
    Tails and everything else are single-gathered."""
    nc = bacc.Bacc("TRN2", target_bir_lowering=False, debug=False,
                   num_devices=NCORES)
    tab = nc.dram_tensor("tab", [N_NODES, D], F32, kind="ExternalInput").ap()
    ncls_rows = 128 * sum(cls_caps)
    tabc = nc.dram_tensor("tabc", [max(ncls_rows, 128), D], F32,
                          kind="ExternalInput").ap()
    idxh = nc.dram_tensor("idxh", [128, F], I32, kind="ExternalInput").ap()
    idxt = nc.dram_tensor("idxt", [128, F], I32, kind="ExternalInput").ap()
    onehot = nc.dram_tensor("onehot", [NG, N_REL, 2048], BF16, kind="ExternalInput").ap()
    u_in = nc.dram_tensor("u", [EP], F32, kind="ExternalInput").ap()
    w1ht = nc.dram_tensor("w1ht", [D, H], F16, kind="ExternalInput").ap()
    w1tt = nc.dram_tensor("w1tt", [D, H], F16, kind="ExternalInput").ap()
    rbt = nc.dram_tensor("rbt", [N_REL, 2 * H], BF16, kind="ExternalInput").ap()  # [hi | lo]
    w2c = nc.dram_tensor("w2c", [96, 2], F32, kind="ExternalInput").ap()
    b2b = nc.dram_tensor("b2b", [128, 1], F32, kind="ExternalInput").ap()
    gate = nc.dram_tensor("gate", [EP], F32, kind="ExternalOutput").ap()

    RELU = mybir.ActivationFunctionType.Relu
    LN = mybir.ActivationFunctionType.Ln
    SIG = mybir.ActivationFunctionType.Sigmoid

    def col_spans(c0, ncols, regions):
        """Split global cols [c0, c0+ncols) into gather spans per the
        (lo, hi, L, kind) regions; everything else is single columns.
        Returns (col, fetch_span, skip) — dup pairs fetch 1, skip 1."""
        spans = []
        c = c0
        end = c0 + ncols
        while c < end:
            sp, fetch = 1, 1
            for (lo, hi, L, kind) in regions:
                if lo <= c < hi and (kind == "seq" or
                                     ((c - lo) % L == 0 and c + L <= end)):
                    if kind == "run":
                        sp = fetch = L
                    elif kind == "seq":
                        sp, fetch = 1, 0   # plain DMA from tabc, no gather
                    elif kind == "tri":
                        sp = fetch = 3   # 4th cell column falls to single
                    else:                # dup
                        sp, fetch = L, 1
                    break
            spans.append((c, fetch, sp))
            c += sp
        return spans

    # head: [16-run region | class regions (kind "seq") | singles]
    head_reg = [(0, 16 * cap16, 16, "run")]
    cls_base = []
    cb = 16 * cap16
    rowbase = 0
    for mi, km in enumerate(cls_caps):
        m = mi + 1
        cls_base.append((cb, m, km, rowbase))
        head_reg.append((cb, cb + m * km, m, "seq"))
        cb += m * km
        rowbase += km
    tail_reg = []

    def class_row(c):
        """table row base for class column c, or None."""
        for (base, m, km, rb) in cls_base:
            if base <= c < base + m * km:
                cell = (c - base) // m
                return 128 * (rb + cell)
        return None

    with tile.TileContext(nc) as tc, contextlib.ExitStack() as ctx:
        constp = ctx.enter_context(tc.tile_pool(name="const", bufs=1))
        gathp = ctx.enter_context(tc.tile_pool(name="gath", bufs=2))
        onep = ctx.enter_context(tc.tile_pool(name="onep", bufs=2))
        xtp = ctx.enter_context(tc.tile_pool(name="xt", bufs=3))
        hps = ctx.enter_context(tc.tile_pool(name="hps", bufs=2, space="PSUM"))
        wps = ctx.enter_context(tc.tile_pool(name="wps", bufs=2, space="PSUM"))
        xpp = ctx.enter_context(tc.tile_pool(name="xpp", bufs=2, space="PSUM"))
        hsbp = ctx.enter_context(tc.tile_pool(name="hsb", bufs=3))
        wsbp = ctx.enter_context(tc.tile_pool(name="wsb", bufs=2))
        finp = ctx.enter_context(tc.tile_pool(name="fin", bufs=1))
        dramp = ctx.enter_context(tc.tile_pool(name="wdram", bufs=1, space="DRAM"))

        idxh_sb = constp.tile([128, F], I32, tag="idxh")
        idxt_sb = constp.tile([128, F], I32, tag="idxt")
        nc.sync.dma_start(out=idxh_sb[:], in_=idxh[:])
        nc.sync.dma_start(out=idxt_sb[:], in_=idxt[:])
        w1ht_sb = constp.tile([D, H], F16, tag="w1ht")
        w1tt_sb = constp.tile([D, H], F16, tag="w1tt")
        rbt_sb = constp.tile([N_REL, 2 * H], BF16, tag="rbt")
        w2c_sb = constp.tile([96, 2], F32, tag="w2c")
        b2b_sb = constp.tile([128, 1], F32, tag="b2b")
        ident = constp.tile([128, 128], F32, tag="ident")
        from concourse.masks import make_identity
        make_identity(nc, ident[:])
        nc.sync.dma_start(out=w1ht_sb[:], in_=w1ht[:])
        nc.sync.dma_start(out=w1tt_sb[:], in_=w1tt[:])
        nc.sync.dma_start(out=rbt_sb[:], in_=rbt[:])
        nc.sync.dma_start(out=w2c_sb[:], in_=w2c[:])
        nc.sync.dma_start(out=b2b_sb[:], in_=b2b[:])

        w_dram = dramp.tile([EP], F32)

        def _emit_w2(p):
            hsb_p, wp_p, s_p, g_p = p
            nch = GCH[g_p]
            nc.tensor.matmul(out=wp_p[32 * s_p:32 * s_p + 1, :],
                             lhsT=w2c_sb[:, 0:1], rhs=hsb_p[:, :512],
                             start=True, stop=False, tile_position=(0, 32 * s_p))
            nc.tensor.matmul(out=wp_p[32 * s_p:32 * s_p + 1, :],
                             lhsT=w2c_sb[:, 1:2], rhs=hsb_p[:, 512:],
                             start=False, stop=True, tile_position=(0, 32 * s_p))
            if s_p == nch - 1:
                w_sb = wsbp.tile([128, 512], F32, tag="wsb")
                nc.vector.tensor_copy(out=w_sb[:], in_=wp_p[:])
                nc.sync.dma_start(
                    out=w_dram[SOFF[g_p]:SOFF[g_p] + nch * 512].rearrange(
                        "(a b) -> a b", a=nch),
                    in_=w_sb[0:32 * nch:32, :])

        # u-dependent half of the gating has no dependency on the MLP:
        # compute ln(eps)-ln(1-eps) during the main pipeline so only
        # w-dependent work remains after the barrier.
        wst = finp.tile([128, F], F32, tag="wst")
        ut = finp.tile([128, F], F32, tag="ut")
        l1 = finp.tile([128, F], F32, tag="l1")
        l2 = finp.tile([128, F], F32, tag="l2")
        gt_ = finp.tile([128, F], F32, tag="gt")
        lnb1 = finp.tile([128, 1], F32, tag="lnb1")
        lnb2 = finp.tile([128, 1], F32, tag="lnb2")
        nc.vector.memset(lnb1[:], float(1.0 - BIAS))
        nc.vector.memset(lnb2[:], float(BIAS))
        nc.sync.dma_start(out=ut[:], in_=u_in[:].rearrange("(p f) -> p f", p=128))
        nc.scalar.activation(out=l1[:], in_=ut[:], func=LN,
                             scale=float(2.0 * BIAS - 1.0), bias=lnb1[:])
        nc.scalar.activation(out=l2[:], in_=ut[:], func=LN,
                             scale=float(1.0 - 2.0 * BIAS), bias=lnb2[:])
        nc.vector.tensor_tensor(out=l1[:], in0=l1[:], in1=l2[:],
                                op=mybir.AluOpType.subtract)

        pending = None
        for g in range(NG):
            ncols = GCOLS[g]
            c0 = COFF[g]
            gh = gathp.tile([128, ncols * D], F32, tag="gh")
            gt = gathp.tile([128, ncols * D], F32, tag="gt")
            for (buf, idx_sb, reg) in ((gh, idxh_sb, head_reg),
                                       (gt, idxt_sb, tail_reg)):
                for (c, fetch, _sp) in col_spans(c0, ncols, reg):
                    j = c - c0
                    if fetch == 0:
                        r0 = class_row(c)
                        nc.sync.dma_start(
                            out=buf[:, j * D:(j + 1) * D],
                            in_=tabc[r0:r0 + 128])
                        continue
                    nc.gpsimd.indirect_dma_start(
                        out=buf[:, j * D:(j + fetch) * D], out_offset=None,
                        in_=tab[:],
                        in_offset=bass.IndirectOffsetOnAxis(
                            ap=idx_sb[:, c:c + 1], axis=0))
            oh = onep.tile([N_REL, ncols * 128], BF16, tag="oh")
            nc.sync.dma_start(out=oh[:], in_=onehot[g][:, :ncols * 128])

            wp = wps.tile([128, 512], F32, tag="wp")
            nc.vector.memset(wp[:], 0.0)
            for s in range(GCH[g]):
                pend = pending
                xpsh = xpp.tile([128, 512], F32, tag="xps")
                xpst = xpp.tile([128, 512], F32, tag="xps")
                for b in range(4):
                    blk = 4 * s + b
                    nc.tensor.transpose(
                        out=xpsh[:, b * 128:(b + 1) * 128],
                        in_=gh[:, blk * D:(blk + 1) * D],
                        identity=ident[:])
                    nc.tensor.transpose(
                        out=xpst[:, b * 128:(b + 1) * 128],
                        in_=gt[:, blk * D:(blk + 1) * D],
                        identity=ident[:])
                xsb = xtp.tile([128, 1024], F16, tag="xsb")
                nc.vector.tensor_copy(out=xsb[:, :512], in_=xpsh[:])
                nc.scalar.activation(out=xsb[:, 512:], in_=xpst[:],
                                     func=mybir.ActivationFunctionType.Copy)
                xh = xsb[:, :512]
                xt_ = xsb[:, 512:]
                hsb = hsbp.tile([96, 1024], F32, tag="hsb")
                for half in range(2):
                    c0h = half * 96
                    hp = hps.tile([96, 512], F32, tag=f"h{half}")
                    nc.tensor.matmul(out=hp[:], lhsT=w1ht_sb[:, c0h:c0h + 96],
                                     rhs=xh[:], start=True, stop=False)
                    nc.tensor.matmul(out=hp[:], lhsT=w1tt_sb[:, c0h:c0h + 96],
                                     rhs=xt_[:], start=False, stop=False)
                    nc.tensor.matmul(out=hp[:], lhsT=rbt_sb[:, c0h:c0h + 96],
                                     rhs=oh[:, s * 512:(s + 1) * 512],
                                     start=False, stop=False)
                    nc.tensor.matmul(out=hp[:], lhsT=rbt_sb[:, H + c0h:H + c0h + 96],
                                     rhs=oh[:, s * 512:(s + 1) * 512],
                                     start=False, stop=True)
                    nc.scalar.activation(out=hsb[:, half * 512:(half + 1) * 512],
                                         in_=hp[:], func=RELU)
                if pend is not None:
                    _emit_w2(pend)
                pending = (hsb, wp, s, g)
        if pending is not None:
            _emit_w2(pending)
            pending = None

        tc.strict_bb_all_engine_barrier()

        # final gating: gate = sigmoid(2*(ln(eps) - ln(1-eps) + w + b2))
        nc.sync.dma_start(out=wst[:], in_=w_dram[:].rearrange("(p f) -> p f", p=128))
        nc.vector.tensor_tensor(out=l1[:], in0=l1[:], in1=wst[:],
                                op=mybir.AluOpType.add)
        nc.scalar.activation(out=gt_[:], in_=l1[:], func=SIG,
                             scale=float(1.0 / TEMP), bias=b2b_sb[:])
        nc.sync.dma_start(out=gate[:].rearrange("(p f) -> p f", p=128), in_=gt_[:])

    nc.compile()
    return nc


def _pos_to_pc():
    """Device output position -> (partition, global col) per slot."""
    pos = np.arange(EP)
    p = np.empty(EP, np.int64)
    c = np.empty(EP, np.int64)
    for g in range(NG):
        base = SOFF[g]
        n = GCH[g] * 512
        r = np.arange(n)
        s, r2 = r // 512, r % 512
        b, pp = r2 // 128, r2 % 128
        p[base:base + n] = pp
        c[base:base + n] = COFF[g] + 4 * s + b
    return p, c


def _match_runs(vals, active, L):
    """Greedy ascending matching of active edge indices into L-tuples
    whose vals are consecutive (v, v+1, ..., v+L-1). Each edge used once."""
    order = np.argsort(vals[active], kind="stable")
    ea = active[order]          # active edges sorted by value
    sv = vals[ea]
    n = len(ea)
    if n == 0:
        return []
    bounds = np.flatnonzero(np.diff(sv)) + 1
    starts = np.concatenate([[0], bounds]).astype(np.int64)
    ends = np.concatenate([bounds, [n]]).astype(np.int64)
    vals_u = sv[starts]
    nxt = starts.copy()         # next unconsumed instance per run
    runs = []
    nr = len(vals_u)
    for r in range(nr - L + 1):
        if not all(r + i < nr and vals_u[r + i] == vals_u[r] + i
                   for i in range(L)):
            continue
        m = min(int(ends[r + i] - nxt[r + i]) for i in range(L))
        for _ in range(m):
            runs.append(tuple(ea[nxt[r + i]] for i in range(L)))
            for i in range(L):
                nxt[r + i] += 1
    return runs


def _prep(edge_index, edge_type, all_embed, relation_emb, u, W1, b1, W2, b2,
          cap16, cls_caps):
    tab32 = np.ascontiguousarray(np.asarray(all_embed, np.float32))
    W1 = np.asarray(W1, np.float32)
    w1ht = np.ascontiguousarray(W1[:, :D].T).astype(np.float16)
    w1tt = np.ascontiguousarray(W1[:, D:2 * D].T).astype(np.float16)
    rb = np.asarray(relation_emb, np.float32) @ W1[:, 2 * D:].T + np.asarray(b1, np.float32)
    import ml_dtypes
    rb_hi = rb.astype(ml_dtypes.bfloat16)
    rb_lo = (rb - rb_hi.astype(np.float32)).astype(ml_dtypes.bfloat16)
    rbt = np.ascontiguousarray(np.concatenate([rb_hi, rb_lo], axis=1))
    W2 = np.asarray(W2, np.float32)
    w2c = np.ascontiguousarray(np.stack([W2[0, :96], W2[0, 96:]], axis=1).astype(np.float32))
    b2b = np.full((128, 1), 2.0 * float(np.asarray(b2).reshape(-1)[0]), np.float32)

    head = np.asarray(edge_index[0], np.int64).astype(np.int64)
    tail = np.asarray(edge_index[1], np.int64).astype(np.int64)
    etype = np.asarray(edge_type, np.int64).astype(np.int64)
    u = np.asarray(u, np.float32)
    pos_p, pos_c = _pos_to_pc()

    in_maps = []
    slot_edge_all = []
    for cidx in range(NCORES):
        sl = slice(cidx * EC, (cidx + 1) * EC)
        h_c, t_c, ty_c, u_c = head[sl], tail[sl], etype[sl], u[sl]

        runs16, cls_members, singles = _layout_core(h_c, cap16, cls_caps)

        # slot table: edge_at[p, c] = edge index or -1
        edge_at = np.full((128, F), -1, np.int64)
        for i, tup in enumerate(runs16):
            blk, p = divmod(i, 128)
            for k in range(16):
                edge_at[p, 16 * blk + k] = tup[k]
        cb = 16 * cap16
        tabc = np.zeros((max(128 * sum(cls_caps), 128), D), np.float32)
        rb = 0
        for mi, groups in enumerate(cls_members):
            m = mi + 1
            km = cls_caps[mi]
            for i, g in enumerate(groups):       # i = cell*128 + p
                cell, p = divmod(i, 128)
                for t in range(m):
                    edge_at[p, cb + m * cell + t] = g[t]
                tabc[128 * (rb + cell) + p] = tab32[h_c[g[0]]]
            cb += m * km
            rb += km
        S0 = cb
        single_col = np.zeros(F, np.bool_)
        single_col[S0:] = True
        free_slots = np.argwhere((edge_at < 0) & single_col[None, :])
        assert len(singles) <= free_slots.shape[0], (
            f"core {cidx}: {len(singles)} singles > {free_slots.shape[0]} slots")
        for k, e in enumerate(singles):
            p, cc = free_slots[k]
            edge_at[p, cc] = e

        # per-slot attribute tables (pads: head/tail 0, type 0, u 0.5)
        valid = edge_at >= 0
        eidx = np.where(valid, edge_at, 0)
        idxh_t = np.where(valid, h_c[eidx], 0).astype(np.int32)
        idxt_t = np.where(valid, t_c[eidx], 0).astype(np.int32)
        ty_t = np.where(valid, ty_c[eidx], 0).astype(np.int64)
        u_t = np.where(valid, u_c[eidx], 0.5).astype(np.float32)

        # self-check: 16-run columns hold consecutive node ids
        for blk in range(cap16):
            for k in range(1, 16):
                assert np.all(idxh_t[:, 16 * blk + k] ==
                              idxh_t[:, 16 * blk] + k)

        # device-position-ordered aux arrays
        t_pos = ty_t[pos_p, pos_c]
        u_dev = u_t[pos_p, pos_c]
        onehot = np.zeros((NG, N_REL, 2048), dtype=ml_dtypes.bfloat16)
        for g in range(NG):
            n = GCH[g] * 512
            tp = t_pos[SOFF[g]:SOFF[g] + n]
            oh = (tp.reshape(1, n) ==
                  np.arange(N_REL, dtype=np.int64).reshape(N_REL, 1))
            onehot[g, :, :n] = oh.astype(ml_dtypes.bfloat16)

        in_maps.append({
            "tab": tab32, "tabc": tabc,
            "idxh": np.ascontiguousarray(idxh_t),
            "idxt": np.ascontiguousarray(idxt_t),
            "onehot": onehot, "u": u_dev,
            "w1ht": w1ht, "w1tt": w1tt, "rbt": rbt, "w2c": w2c, "b2b": b2b,
        })
        slot_edge_all.append(edge_at)
    return in_maps, slot_edge_all, pos_p, pos_c


def _match_same(vals, active):
    """Greedy pairing of active edge indices sharing the same value."""
    order = np.argsort(vals[active], kind="stable")
    ea = active[order]
    sv = vals[ea]
    pairs = []
    i = 0
    n = len(ea)
    while i + 1 < n:
        if sv[i] == sv[i + 1]:
            pairs.append((ea[i], ea[i + 1]))
            i += 2
        else:
            i += 1
    return pairs


def _layout_core(h_c, cap16, cls_caps):
    """v4: 16-run matching on heads, then multiplicity-class cells
    (m=2..9) on the residual. Returns (runs16, cls_members, singles)."""
    all_e = np.arange(EC)
    used = np.zeros(EC, np.bool_)
    runs16 = _match_runs(h_c, all_e, 16)[:cap16 * 128]
    for tup in runs16:
        for e in tup:
            used[e] = True
    rem_idx = all_e[~used]
    order = np.argsort(h_c[rem_idx], kind="stable")
    rem_sorted = rem_idx[order]
    vals = h_c[rem_sorted]
    bounds = np.flatnonzero(np.diff(vals)) + 1
    starts = np.concatenate([[0], bounds]).astype(np.int64)
    ends = np.concatenate([bounds, [len(vals)]]).astype(np.int64)
    cnts = ends - starts
    # values sorted by count desc; class-m (m=9..2) consumes the next
    # 128*K_m values, using m edges of each (leftovers -> singles)
    vorder = np.argsort(-cnts, kind="stable")
    cls_members = [None] * len(cls_caps)
    pos = 0
    for m in range(16, 0, -1):
        km = cls_caps[m - 1]
        groups = []
        for vi in vorder[pos:pos + 128 * km]:
            s = int(starts[vi])
            assert int(cnts[vi]) >= m
            groups.append(rem_sorted[s:s + m])
        pos += 128 * km
        for g in groups:
            for e in g:
                used[e] = True
        cls_members[m - 1] = groups
    return runs16, cls_members, all_e[~used]


def _pair_config(edge_index):
    """Shared-across-cores caps: 16-run blocks + class-cell counts."""
    head = np.asarray(edge_index[0], np.int64)
    hs = [head[c * EC:(c + 1) * EC] for c in range(NCORES)]
    # classes beat 16-runs per column (1.0 vs 1.0625 Pool) and their
    # supply is richer without run-matching competing for the same
    # dup-heavy values: head side is all-class.
    cap16 = 0
    core_cnts = []
    for h in hs:
        _vals, cnts = np.unique(h, return_counts=True)
        core_cnts.append(cnts)
    # demotion ladder: class-m cells accept any value with count >= m,
    # filled from m=9 downward so no per-class 128-floor waste.
    used = [0] * NCORES
    caps_desc = []
    for m in range(16, 0, -1):
        avail = [int((core_cnts[c] >= m).sum()) - used[c]
                 for c in range(NCORES)]
        km = max(min(avail), 0) // 128
        caps_desc.append(km)
        for c in range(NCORES):
            used[c] += 128 * km
    cls_caps = list(reversed(caps_desc))   # index mi -> m = mi + 1
    # column budget: 16*cap16 + sum(m*K_m) <= F; trim largest classes first
    while 16 * cap16 + sum((i + 1) * k
                           for i, k in enumerate(cls_caps)) > F:
        for i in range(15, -1, -1):
            if cls_caps[i] > 0:
                cls_caps[i] -= 1
                break
        else:
            break
    return cap16, tuple(cls_caps)


def kernel(edge_index, edge_type, all_embed, relation_emb, u, W1, b1, W2, b2):
    # Shard-by-sorted-head: concentrating each core's head values into
    # ~1/8 of the node range multiplies the consecutive/duplicate run
    # supply. The pipeline is permutation-equivariant: reorder edges at
    # entry, un-permute the outputs at exit.
    edge_index = np.asarray(edge_index)
    order = np.argsort(edge_index[0], kind="stable")
    ei2 = edge_index[:, order]
    et2 = np.asarray(edge_type)[order]
    u2 = np.asarray(u)[order]
    if "nc" not in _CACHE:
        _CACHE["cfg"] = _pair_config(ei2)
        _CACHE["nc"] = _build_program(*_CACHE["cfg"])
    nc = _CACHE["nc"]
    in_maps, slot_edge_all, pos_p, pos_c = _prep(
        ei2, et2, all_embed, relation_emb, u2, W1, b1, W2, b2,
        *_CACHE["cfg"])
    res = run_bass_kernel_spmd(nc, in_maps, list(range(NCORES)))
    out = np.empty(E, np.float32)
    for cidx in range(NCORES):
        gate_pos = res.results[cidx]["gate"]          # [EP] in pos order
        edge_at = slot_edge_all[cidx]
        e_of_pos = edge_at[pos_p, pos_c]              # edge or -1 per pos
        m = e_of_pos >= 0
        out_core = np.empty(EC, np.float32)
        out_core[e_of_pos[m]] = gate_pos[m]
        out[cidx * EC:(cidx + 1) * EC] = out_core
    unperm = np.empty(E, np.float32)
    unperm[order] = out
    return unperm

